# revision 3
# baseline (speedup 1.0000x reference)
"""Trainium2 Bass kernel for GCN(x2) + MHA + mean + FC, sharded over 8 NeuronCores.

Sharding: 1D row partition of the 4096 nodes (512 rows/core). Each core holds
the column slice adj_hat[:, r*512:(r+1)*512] of the symmetric A+I (by symmetry
equal to its row block transposed), all of x, and replicated weights.
Cross-core exchanges (on-device AllGather): degree vector, GCN1 output, K/V.
All activations are kept in [feature, node] layout so no transposes are needed.
Matmuls run in bf16 (the adjacency is binary, so exact) with fp32 PSUM accum.
Host does only slicing (shard) and an 8-way sum of [2]-vector partials (gather).
"""
import sys
sys.path.insert(0, "/opt/trn_rl_repo")
import numpy as np

N = 4096
NC_ = 8
R = N // NC_          # 512 rows per core
KB = N // 128         # 32 node chunks
F_IN = 128
G1 = 128
G2 = 512
HEADS = 4
HD = G2 // HEADS      # 128
ET = G2 // 128        # 4 tiles of the 512-dim embedding

_cache = {}


def _build():
    from concourse import bass, bacc, tile, mybir

    f32 = mybir.dt.float32
    bf16 = mybir.dt.bfloat16
    AF = mybir.ActivationFunctionType
    ALU = mybir.AluOpType
    AX = mybir.AxisListType

    nc = bacc.Bacc("TRN2", target_bir_lowering=False, debug=False,
                   num_devices=NC_)

    # ---- kernel I/O (per-core shards supplied via in_maps) ----
    adj_d = nc.dram_tensor("adjc", [N, R], f32, kind="ExternalInput")
    x_d = nc.dram_tensor("x", [N, F_IN], f32, kind="ExternalInput")
    w1_d = nc.dram_tensor("w1", [F_IN, G1], f32, kind="ExternalInput")
    b1_d = nc.dram_tensor("b1", [G1], f32, kind="ExternalInput")
    w2_d = nc.dram_tensor("w2", [G1, G2], f32, kind="ExternalInput")
    b2_d = nc.dram_tensor("b2", [G2], f32, kind="ExternalInput")
    win_d = nc.dram_tensor("win", [G2, 3 * G2], f32, kind="ExternalInput")
    bin_d = nc.dram_tensor("bin", [3 * G2], f32, kind="ExternalInput")
    wo_d = nc.dram_tensor("wo", [G2, G2], f32, kind="ExternalInput")
    bo_d = nc.dram_tensor("bo", [G2], f32, kind="ExternalInput")
    fcw_d = nc.dram_tensor("fcw", [G2, 2], f32, kind="ExternalInput")
    fcb_d = nc.dram_tensor("fcb", [2], f32, kind="ExternalInput")
    out_d = nc.dram_tensor("outp", [1, 2], f32, kind="ExternalOutput")

    RG = [list(range(NC_))]

    with tile.TileContext(nc) as tc:
        with tc.tile_pool(name="wts", bufs=1) as wts, \
             tc.tile_pool(name="adj", bufs=1) as adjp, \
             tc.tile_pool(name="stage", bufs=3) as stg, \
             tc.tile_pool(name="wstage", bufs=2) as wstg_p, \
             tc.tile_pool(name="xs", bufs=1) as xsp, \
             tc.tile_pool(name="x1s", bufs=1) as x1sp, \
             tc.tile_pool(name="act", bufs=1) as actp, \
             tc.tile_pool(name="kvq", bufs=1) as kvp, \
             tc.tile_pool(name="ktl", bufs=2) as ktlp, \
             tc.tile_pool(name="ktg", bufs=16) as ktgp, \
             tc.tile_pool(name="vv", bufs=40) as vvp, \
             tc.tile_pool(name="pt", bufs=3) as ptp, \
             tc.tile_pool(name="small", bufs=2) as smp, \
             tc.tile_pool(name="psA", bufs=3, space="PSUM") as psA, \
             tc.tile_pool(name="psB", bufs=2, space="PSUM") as psB, \
             tc.tile_pool(name="psC", bufs=2, space="PSUM") as psC, \
             tc.tile_pool(name="dram", bufs=1, space="DRAM") as drp:

            # ================= constants / weights =================
            ones_b = wts.tile([128, 1], bf16)
            nc.vector.memset(ones_b[:], 1.0)
            ones_f = wts.tile([128, 1], f32)
            nc.vector.memset(ones_f[:], 1.0)

            # W1 [128,128] -> bf16
            w1_f = wstg_p.tile([128, G1], f32, tag="wstg")
            nc.sync.dma_start(w1_f[:], w1_d[:, :])
            w1_b = wts.tile([128, G1], bf16)
            nc.gpsimd.tensor_copy(w1_b[:], w1_f[:])
            # W2 [128,512] -> bf16
            w2_f = wstg_p.tile([128, G2], f32, tag="wstg")
            nc.sync.dma_start(w2_f[:], w2_d[:, :])
            w2_b = wts.tile([128, G2], bf16)
            nc.gpsimd.tensor_copy(w2_b[:], w2_f[:])
            # in_proj [512,1536] -> 4 bf16 tiles [128,1536]
            win_b = []
            for c in range(ET):
                wf = wstg_p.tile([128, 3 * G2], f32, tag="winstg")
                nc.sync.dma_start(wf[:], win_d[c * 128:(c + 1) * 128, :])
                wb = wts.tile([128, 3 * G2], bf16, tag=f"winb{c}")
                nc.gpsimd.tensor_copy(wb[:], wf[:])
                win_b.append(wb)
            # out_proj fp32 4 tiles [128,512]
            wo_f = []
            for c in range(ET):
                wf = wts.tile([128, G2], f32, tag=f"wo{c}")
                nc.sync.dma_start(wf[:], wo_d[c * 128:(c + 1) * 128, :])
                wo_f.append(wf)
            # fc_w 4 tiles [128,2]
            fcw_f = []
            for c in range(ET):
                wf = wts.tile([128, 2], f32, tag=f"fcw{c}")
                nc.sync.dma_start(wf[:], fcw_d[c * 128:(c + 1) * 128, :])
                fcw_f.append(wf)
            # biases
            b1_row = wts.tile([1, G1], f32)
            nc.sync.dma_start(b1_row[:], b1_d[:])
            b1_bc = wts.tile([128, G1], f32)
            nc.gpsimd.partition_broadcast(b1_bc[:], b1_row[:])
            b2_col = wts.tile([128, ET], f32)
            for c in range(ET):
                nc.sync.dma_start(b2_col[:, c:c + 1], b2_d[c * 128:(c + 1) * 128])
            bin_col = wts.tile([128, 12], f32)
            for c in range(12):
                nc.sync.dma_start(bin_col[:, c:c + 1], bin_d[c * 128:(c + 1) * 128])
            bo8_col = wts.tile([128, ET], f32)
            for c in range(ET):
                nc.sync.dma_start(bo8_col[:, c:c + 1], bo_d[c * 128:(c + 1) * 128])
            bo8s = wts.tile([128, ET], f32)
            nc.vector.tensor_scalar_mul(bo8s[:], bo8_col[:], 1.0 / NC_)
            fcb_row = wts.tile([1, 2], f32)
            nc.sync.dma_start(fcb_row[:], fcb_d[:])
            fcb8 = wts.tile([1, 2], f32)
            nc.vector.tensor_scalar_mul(fcb8[:], fcb_row[:], 1.0 / NC_)

            # ================= A: adjacency load + degree =================
            adj_sb = []
            ps_deg = psC.tile([1, G2], f32, tag="sm")
            for kb in range(KB):
                af = stg.tile([128, R], f32, tag="adjstg")
                nc.sync.dma_start(af[:], adj_d[kb * 128:(kb + 1) * 128, :])
                ab = adjp.tile([128, R], bf16, tag=f"adj{kb}")
                nc.gpsimd.tensor_copy(ab[:], af[:])
                adj_sb.append(ab)
                nc.tensor.matmul(ps_deg[:], ones_b[:], ab[:],
                                 start=(kb == 0), stop=(kb == KB - 1))
            # d_local = 1/sqrt(deg)  [1,512]
            sq = smp.tile([1, G2], f32, tag="sq")
            nc.scalar.activation(sq[:], ps_deg[:], AF.Sqrt)
            dloc = wts.tile([1, G2], f32)
            nc.vector.reciprocal(dloc[:], sq[:])

            # AG1: gather d across cores -> d for all 4096 nodes
            dg_in = drp.tile([1, G2], f32, tag="dgin")
            dg_out = drp.tile([NC_, G2], f32, tag="dgout")
            nc.sync.dma_start(dg_in[:], dloc[:])
            nc.gpsimd.collective_compute(
                "AllGather", ALU.bypass, replica_groups=RG,
                ins=[dg_in.opt()], outs=[dg_out.opt()])
            dcol = wts.tile([128, KB], f32)
            for kb in range(KB):
                rr, jb = kb // 4, kb % 4
                nc.sync.dma_start(dcol[:, kb:kb + 1],
                                  dg_out[rr:rr + 1, jb * 128:(jb + 1) * 128])
            # broadcast of own-row scale for free-dim scaling [128,512]
            dbc = wts.tile([128, G2], f32)
            nc.gpsimd.partition_broadcast(dbc[:], dloc[:])

            # ================= B: x load/scale + GCN1 =================
            xs_sb = []
            for kb in range(KB):
                xf = stg.tile([128, F_IN], f32, tag="xstg")
                nc.sync.dma_start(xf[:], x_d[kb * 128:(kb + 1) * 128, :])
                xb = xsp.tile([128, F_IN], bf16, tag=f"xs{kb}")
                nc.vector.tensor_scalar_mul(xb[:], xf[:], dcol[:, kb:kb + 1])
                xs_sb.append(xb)
            ps_s1 = psA.tile([128, R], f32, tag="big")
            for kb in range(KB):
                nc.tensor.matmul(ps_s1[:], xs_sb[kb][:], adj_sb[kb][:],
                                 start=(kb == 0), stop=(kb == KB - 1))
            s1t = actp.tile([128, R], bf16, tag="s1t")
            nc.vector.tensor_mul(s1t[:], ps_s1[:], dbc[:])
            # x1 = relu(s1.T @ W1 + b1), natural [node, g] 4 tiles
            x1_sb = []
            for mt in range(ET):
                psx = psC.tile([128, G1], f32, tag="sm")
                nc.tensor.matmul(psx[:], s1t[:, mt * 128:(mt + 1) * 128],
                                 w1_b[:], start=True, stop=True)
                tmp = smp.tile([128, G1], f32, tag="x1tmp")
                nc.vector.tensor_add(tmp[:], psx[:], b1_bc[:])
                xb = actp.tile([128, G1], bf16, tag=f"x1_{mt}")
                nc.scalar.activation(xb[:], tmp[:], AF.Relu)
                x1_sb.append(xb)

            # AG2: gather x1 (bf16, natural [node,g])
            x1_in = drp.tile([R, G1], bf16, tag="x1in")
            x1_out = drp.tile([N, G1], bf16, tag="x1out")
            for mt in range(ET):
                nc.sync.dma_start(x1_in[mt * 128:(mt + 1) * 128, :], x1_sb[mt][:])
            nc.gpsimd.collective_compute(
                "AllGather", ALU.bypass, replica_groups=RG,
                ins=[x1_in.opt()], outs=[x1_out.opt()])

            # ================= C: GCN2 =================
            x1s_sb = []
            for kb in range(KB):
                xg = x1sp.tile([128, G1], bf16, tag=f"x1s{kb}")
                nc.sync.dma_start(xg[:], x1_out[kb * 128:(kb + 1) * 128, :])
                xsc = x1sp.tile([128, G1], bf16, tag=f"x1sc{kb}")
                nc.vector.tensor_scalar_mul(xsc[:], xg[:], dcol[:, kb:kb + 1])
                x1s_sb.append(xsc)
            ps_s2 = psA.tile([128, R], f32, tag="big")
            for kb in range(KB):
                nc.tensor.matmul(ps_s2[:], x1s_sb[kb][:], adj_sb[kb][:],
                                 start=(kb == 0), stop=(kb == KB - 1))
            s2t = actp.tile([128, R], bf16, tag="s2t")
            nc.vector.tensor_mul(s2t[:], ps_s2[:], dbc[:])
            # x2T tiles [e-tile 128, m 512], bias per-partition
            x2t_sb = []
            for et in range(ET):
                psx = psA.tile([128, R], f32, tag="big")
                nc.tensor.matmul(psx[:], w2_b[:, et * 128:(et + 1) * 128],
                                 s2t[:], start=True, stop=True)
                xt = actp.tile([128, R], bf16, tag=f"x2_{et}")
                nc.scalar.activation(xt[:], psx[:], AF.Identity,
                                     bias=b2_col[:, et:et + 1])
                x2t_sb.append(xt)

            # ================= D: QKV + AG3 per head =================
            qt_sb = {}
            kv_out = {}
            for h in range(HEADS):
                # QT_h [d,512] = Wq_h.T @ x2T + bq
                psq = psA.tile([128, R], f32, tag="big")
                for c in range(ET):
                    nc.tensor.matmul(psq[:], win_b[c][:, h * 128:(h + 1) * 128],
                                     x2t_sb[c][:], start=(c == 0), stop=(c == ET - 1))
                qt = kvp.tile([128, R], bf16, tag=f"qt{h}")
                nc.scalar.activation(qt[:], psq[:], AF.Identity,
                                     bias=bin_col[:, h:h + 1])
                qt_sb[h] = qt
                # KT_h
                psk = psA.tile([128, R], f32, tag="big")
                for c in range(ET):
                    nc.tensor.matmul(psk[:], win_b[c][:, G2 + h * 128:G2 + (h + 1) * 128],
                                     x2t_sb[c][:], start=(c == 0), stop=(c == ET - 1))
                kt = ktlp.tile([128, R], bf16, tag="ktloc")
                nc.scalar.activation(kt[:], psk[:], AF.Identity,
                                     bias=bin_col[:, 4 + h:5 + h])
                # V_h natural [node,d] 4 tiles (bias folded in post-norm)
                vloc = []
                for mt in range(ET):
                    psv = psC.tile([128, HD], f32, tag="sm")
                    for c in range(ET):
                        nc.tensor.matmul(
                            psv[:],
                            x2t_sb[c][:, mt * 128:(mt + 1) * 128],
                            win_b[c][:, 2 * G2 + h * 128:2 * G2 + (h + 1) * 128],
                            start=(c == 0), stop=(c == ET - 1))
                    vb = ptp.tile([128, HD], bf16, tag="vloc")
                    nc.vector.tensor_copy(vb[:], psv[:])
                    vloc.append(vb)
                # bounce + AllGather (rows: rank block = [KT(128) ; vpack(128)])
                kvi = drp.tile([256, R], bf16, tag=f"kvi{h}")
                kvo = drp.tile([NC_ * 256, R], bf16, tag=f"kvo{h}")
                nc.sync.dma_start(kvi[0:128, :], kt[:])
                for mt in range(ET):
                    nc.sync.dma_start(kvi[128:256, mt * 128:(mt + 1) * 128],
                                      vloc[mt][:])
                nc.gpsimd.collective_compute(
                    "AllGather", ALU.bypass, replica_groups=RG,
                    ins=[kvi.opt()], outs=[kvo.opt()])
                kv_out[h] = kvo

            # ================= E: attention per head =================
            z_sb = []
            inv_sqrt_hd = 1.0 / float(np.sqrt(HD))
            for h in range(HEADS):
                kvo = kv_out[h]
                # load gathered KT (8 x [128,512]) and V (32 x [128,128])
                kts = []
                vs = []
                for rr in range(NC_):
                    ktile = ktgp.tile([128, R], bf16, tag="ktg")
                    nc.sync.dma_start(ktile[:], kvo[rr * 256:rr * 256 + 128, :])
                    kts.append(ktile)
                    for mt in range(ET):
                        vt = vvp.tile([128, HD], bf16, tag="vg")
                        nc.sync.dma_start(
                            vt[:], kvo[rr * 256 + 128:rr * 256 + 256,
                                       mt * 128:(mt + 1) * 128])
                        vs.append(vt)
                ps_ctx = psB.tile([128, R], f32, tag="ctx")
                acc = smp.tile([128, R], f32, tag="acc")
                for kc in range(KB):
                    rr, lb = kc // 4, kc % 4
                    ps_sc = psA.tile([128, R], f32, tag="big")
                    nc.tensor.matmul(ps_sc[:],
                                     kts[rr][:, lb * 128:(lb + 1) * 128],
                                     qt_sb[h][:], start=True, stop=True,
                                     skip_group_check=True)
                    pt = ptp.tile([128, R], bf16, tag="pt")
                    nc.scalar.activation(pt[:], ps_sc[:], AF.Exp,
                                         scale=inv_sqrt_hd)
                    if kc == 0:
                        nc.vector.tensor_copy(acc[:], pt[:])
                    else:
                        nc.vector.tensor_add(acc[:], acc[:], pt[:])
                    nc.tensor.matmul(ps_ctx[:], vs[kc][:], pt[:],
                                     start=(kc == 0), stop=(kc == KB - 1),
                                     skip_group_check=True)
                # denominator and normalized row-sum
                ps_den = psC.tile([1, R], f32, tag="sm")
                nc.tensor.matmul(ps_den[:], ones_f[:], acc[:],
                                 start=True, stop=True, skip_group_check=True)
                rden = smp.tile([1, R], f32, tag="rden")
                nc.vector.reciprocal(rden[:], ps_den[:])
                rbc = smp.tile([128, R], f32, tag="rbc")
                nc.gpsimd.partition_broadcast(rbc[:], rden[:])
                ctxn = smp.tile([128, R], f32, tag="ctxn")
                nc.vector.tensor_mul(ctxn[:], ps_ctx[:], rbc[:])
                zs = smp.tile([128, 1], f32, tag=f"z{h}")
                nc.vector.tensor_reduce(zs[:], ctxn[:], axis=AX.X, op=ALU.add)
                # fold V-bias: + R_local_rows * bv ... careful: sum over own 512
                # queries of constant bv -> 512*bv
                zb = smp.tile([128, 1], f32, tag=f"zb{h}")
                nc.vector.tensor_scalar_mul(zb[:], bin_col[:, 8 + h:9 + h],
                                            float(R))
                zf = smp.tile([128, 1], f32, tag=f"zf{h}")
                nc.vector.tensor_add(zf[:], zs[:], zb[:])
                z_sb.append(zf)

            # ================= F: out_proj + mean + fc (partial) =================
            u_sb = []
            for et in range(ET):
                psu = psC.tile([128, 1], f32, tag="sm")
                for c in range(ET):
                    nc.tensor.matmul(psu[:], wo_f[c][:, et * 128:(et + 1) * 128],
                                     z_sb[c][:], start=(c == 0), stop=(c == ET - 1))
                ut = smp.tile([128, 1], f32, tag=f"u{et}")
                nc.scalar.activation(ut[:], psu[:], AF.Identity,
                                     scale=1.0 / float(N),
                                     bias=bo8s[:, et:et + 1])
                u_sb.append(ut)
            ps_fc = psC.tile([1, 2], f32, tag="sm")
            for c in range(ET):
                nc.tensor.matmul(ps_fc[:], u_sb[c][:], fcw_f[c][:],
                                 start=(c == 0), stop=(c == ET - 1))
            ores = smp.tile([1, 2], f32, tag="ores")
            nc.vector.tensor_add(ores[:], ps_fc[:], fcb8[:])
            nc.sync.dma_start(out_d[:, :], ores[:])

    nc.compile()
    return nc


def kernel(**inputs):
    from concourse.bass_utils import run_bass_kernel_spmd

    if "nc" not in _cache:
        _cache["nc"] = _build()
    nc = _cache["nc"]

    adj = np.ascontiguousarray(inputs["adj_matrix"], dtype=np.float32)
    x = np.ascontiguousarray(inputs["node_features"], dtype=np.float32)
    reps = {
        "x": x,
        "w1": np.ascontiguousarray(inputs["W1"], np.float32),
        "b1": np.ascontiguousarray(inputs["b1"], np.float32),
        "w2": np.ascontiguousarray(inputs["W2"], np.float32),
        "b2": np.ascontiguousarray(inputs["b2"], np.float32),
        "win": np.ascontiguousarray(inputs["in_proj_w"], np.float32),
        "bin": np.ascontiguousarray(inputs["in_proj_b"], np.float32),
        "wo": np.ascontiguousarray(inputs["out_proj_w"], np.float32),
        "bo": np.ascontiguousarray(inputs["out_proj_b"], np.float32),
        "fcw": np.ascontiguousarray(inputs["fc_w"], np.float32),
        "fcb": np.ascontiguousarray(inputs["fc_b"], np.float32),
    }
    in_maps = []
    idx = np.arange(R)
    for r in range(NC_):
        cols = np.ascontiguousarray(adj[:, r * R:(r + 1) * R])
        cols[r * R + idx, idx] += 1.0   # A + I, this core's diagonal block
        in_maps.append({"adjc": cols, **reps})

    res = run_bass_kernel_spmd(nc, in_maps, core_ids=list(range(NC_)))
    out = np.zeros(2, dtype=np.float64)
    for r in range(NC_):
        out += res.results[r]["outp"].reshape(2).astype(np.float64)
    return out.astype(np.float32)
